# revision 1
# baseline (speedup 1.0000x reference)
"""Trainium2 Bass kernel for the EighMSE loss (data-parallel over 8 cores).

Math (replicates jax/LAPACK ssyevd eigenvector sign conventions for 2x2
symmetric matrices):
  row (a, b, c) encodes [[a, b], [b, c]]
  SM = a + c, DF = a - c, RT = sqrt(DF^2 + 4 b^2)
  closed-form evals = (SM +- RT) / 2
  x = clamp(DF / RT, -1, 1)
  n0 = sqrt((1 + x)/2) >= 0,  n1u = sqrt((1 - x)/2) >= 0
  LAPACK larger-eval eigenvector g = (tau0 * n0, tau1 * n1u) with
    tau0 = -1 if DF > 0 else sign(b) * sign(SM)
    tau1 = tau0 * sign(b)
  smaller-eval eigenvector = sign(SM) * (-g1, g0)

Per-core partial sums (10 f32 values per partition, summed on host):
  A  = sum dSM^2      Bs = sum dRT^2     C = sum dDF^2     D = sum db^2
  E1 = sum dg0^2      E2 = sum dg1^2
  SP0 = sum g0p*g0t   SP1 = sum g1p*g1t
  SP0m = sum sx*g0p*g0t   SP1m = sum sx*g1p*g1t  (sx = [sign(SMp) != sign(SMt)])
Host combine:
  F0 = E1 + 4*SP0m ; F1 = E2 + 4*SP1m
  loss = w0*(A+Bs)/(4B) + w1*E1/B + w2*E2/B + w3*F1/B + w4*F0/B
         + w5*(A/2 + C/2 + D)/(3B)
"""

import numpy as np
from contextlib import ExitStack

import concourse.bass as bass
import concourse.bacc as bacc
import concourse.tile as tile
from concourse import mybir
from concourse.bass_utils import run_bass_kernel_spmd

F32 = mybir.dt.float32
BF16 = mybir.dt.bfloat16
OP = mybir.AluOpType
AF = mybir.ActivationFunctionType

B_TOTAL = 4_194_304
NCORES = 8
S = B_TOTAL // NCORES          # samples per core
P = 128                        # partitions
NPC = S // P                   # samples per partition (4096)
W = 1024                       # samples per tile per partition
NT = NPC // W                  # tiles per core
NTERMS = 10

_BUILT = None


def _build_bass():
    nc = bacc.Bacc()
    yp = nc.declare_dram_parameter("y_pred", [S, 3], F32, isOutput=False)
    yt = nc.declare_dram_parameter("y_true", [S, 3], F32, isOutput=False)
    out = nc.declare_dram_parameter("out", [P, NTERMS], F32, isOutput=True)

    ypr = yp.rearrange("(p n) c -> p n c", p=P)
    ytr = yt.rearrange("(p n) c -> p n c", p=P)

    with tile.TileContext(nc) as tc, ExitStack() as ctx:
        inp = ctx.enter_context(tc.tile_pool(name="inp", bufs=2))
        wk = ctx.enter_context(tc.tile_pool(name="wk", bufs=1))
        bk = ctx.enter_context(tc.tile_pool(name="bk", bufs=1))
        dp = ctx.enter_context(tc.tile_pool(name="dp", bufs=2))
        accp = ctx.enter_context(tc.tile_pool(name="accp", bufs=1))

        stats = accp.tile([P, NTERMS * NT], F32)
        nc.vector.memset(stats[:], 0.0)
        halfc = accp.tile([P, 1], F32)
        nc.vector.memset(halfc[:], 0.5)

        def tensor_chain(x, pref):
            a = x[:, :, 0]
            b = x[:, :, 1]
            c = x[:, :, 2]

            SM = wk.tile([P, W], F32, tag=pref + "SM")
            nc.vector.tensor_add(SM[:], a, c)
            DF = wk.tile([P, W], F32, tag=pref + "DF")
            nc.vector.tensor_sub(DF[:], a, c)

            sq1 = wk.tile([P, W], F32, tag=pref + "sq1")   # TB2 -> RT2
            nc.scalar.activation(sq1[:], b, AF.Square, scale=2.0)
            sq2 = wk.tile([P, W], F32, tag=pref + "sq2")   # DF2 -> RT
            nc.scalar.activation(sq2[:], DF[:], AF.Square)
            nc.vector.tensor_add(sq1[:], sq2[:], sq1[:])          # RT2 (in-place)
            nc.scalar.activation(sq2[:], sq1[:], AF.Sqrt)         # RT
            RT = sq2

            r = wk.tile([P, W], F32, tag=pref + "r")       # r -> x -> xc
            nc.vector.reciprocal_approx_fast(r[:], RT[:])
            nc.vector.tensor_mul(r[:], DF[:], r[:])               # x (in-place)
            nc.vector.tensor_scalar(r[:], r[:], 1.0, -1.0, op0=OP.min, op1=OP.max)
            xc = r

            n0 = bk.tile([P, W], BF16, tag=pref + "n0")
            nc.scalar.activation(n0[:], xc[:], AF.Sqrt, bias=halfc[:], scale=0.5)
            n1u = bk.tile([P, W], BF16, tag=pref + "n1u")
            nc.scalar.activation(n1u[:], xc[:], AF.Sqrt, bias=halfc[:], scale=-0.5)

            # masks (1/0 in bf16): mb = b<0, ms = SM<0, mDF = DF>0
            mb = bk.tile([P, W], BF16, tag=pref + "mb")    # mb -> q1 -> t1 -> g1
            nc.vector.tensor_single_scalar(mb[:], b, 0.0, op=OP.is_lt)
            ms = bk.tile([P, W], BF16, tag=pref + "ms")
            nc.vector.tensor_single_scalar(ms[:], SM[:], 0.0, op=OP.is_lt)
            mDF = bk.tile([P, W], BF16, tag=pref + "mDF")
            nc.vector.tensor_single_scalar(mDF[:], DF[:], 0.0, op=OP.is_gt)

            # q0 = [tau0<0] = mDF OR (mb XOR ms) ; q1 = q0 XOR mb
            mg = bk.tile([P, W], BF16, tag=pref + "mg")    # mneg -> q0 -> t0 -> g0
            nc.vector.tensor_tensor(mg[:], mb[:], ms[:], op=OP.not_equal)
            nc.vector.tensor_max(mg[:], mDF[:], mg[:])            # q0 (in-place)
            nc.vector.tensor_tensor(mb[:], mg[:], mb[:], op=OP.not_equal)  # q1
            # tau = 1 - 2q
            nc.vector.tensor_scalar(mg[:], mg[:], -2.0, 1.0, op0=OP.mult, op1=OP.add)
            nc.vector.tensor_scalar(mb[:], mb[:], -2.0, 1.0, op0=OP.mult, op1=OP.add)
            # g = tau * n
            nc.vector.tensor_mul(mg[:], mg[:], n0[:])             # g0
            nc.vector.tensor_mul(mb[:], mb[:], n1u[:])            # g1

            return dict(SM=SM[:], RT=RT[:], DF=DF[:], b=b, g0=mg[:], g1=mb[:], ms=ms[:])

        scr = accp.tile([P, W], F32)

        def sq_acc(d_ap, col):
            nc.scalar.activation(scr[:], d_ap, AF.Square, accum_out=stats[:, col : col + 1])

        def cp_acc(d_ap, col):
            nc.scalar.activation(scr[:], d_ap, AF.Copy, accum_out=stats[:, col : col + 1])

        for i in range(NT):
            xp = inp.tile([P, W, 3], F32, tag="xp")
            nc.sync.dma_start(xp[:], ypr[:, bass.ts(i, W), :])
            xt = inp.tile([P, W, 3], F32, tag="xt")
            nc.sync.dma_start(xt[:], ytr[:, bass.ts(i, W), :])

            tp = tensor_chain(xp, "p_")
            tt = tensor_chain(xt, "t_")

            # linear / eigenvalue diff terms (f32)
            for k, name in enumerate(["SM", "RT", "DF", "b"]):
                d = dp.tile([P, W], F32, tag="d")
                nc.vector.tensor_sub(d[:], tp[name], tt[name])
                sq_acc(d[:], k * NT + i)

            # eigenvector diff terms (bf16)
            dg0 = dp.tile([P, W], BF16, tag="dg0")
            nc.vector.tensor_sub(dg0[:], tp["g0"], tt["g0"])
            sq_acc(dg0[:], 4 * NT + i)
            dg1 = dp.tile([P, W], BF16, tag="dg1")
            nc.vector.tensor_sub(dg1[:], tp["g1"], tt["g1"])
            sq_acc(dg1[:], 5 * NT + i)

            # cross products for the sign(SM)-flipped terms
            sx = dp.tile([P, W], BF16, tag="sx")
            nc.vector.tensor_tensor(sx[:], tp["ms"], tt["ms"], op=OP.not_equal)
            P0 = dp.tile([P, W], BF16, tag="P0")
            nc.vector.tensor_mul(P0[:], tp["g0"], tt["g0"])
            P1 = dp.tile([P, W], BF16, tag="P1")
            nc.vector.tensor_mul(P1[:], tp["g1"], tt["g1"])
            P0m = dp.tile([P, W], BF16, tag="P0m")
            nc.vector.tensor_mul(P0m[:], P0[:], sx[:])
            P1m = dp.tile([P, W], BF16, tag="P1m")
            nc.vector.tensor_mul(P1m[:], P1[:], sx[:])
            cp_acc(P0[:], 6 * NT + i)
            cp_acc(P1[:], 7 * NT + i)
            cp_acc(P0m[:], 8 * NT + i)
            cp_acc(P1m[:], 9 * NT + i)

        outsums = accp.tile([P, NTERMS], F32)
        stats3 = stats[:].rearrange("p (t i) -> p t i", t=NTERMS)
        for t in range(NTERMS):
            nc.vector.tensor_reduce(
                outsums[:, t : t + 1], stats3[:, t, :], axis=mybir.AxisListType.X, op=OP.add
            )
        nc.sync.dma_start(out[:, :], outsums[:])

    nc.compile()
    return nc


def _get_built():
    global _BUILT
    if _BUILT is None:
        _BUILT = _build_bass()
    return _BUILT


def kernel(y_pred: np.ndarray, y_true: np.ndarray, weights: np.ndarray) -> np.ndarray:
    y_pred = np.ascontiguousarray(y_pred, dtype=np.float32)
    y_true = np.ascontiguousarray(y_true, dtype=np.float32)
    w = np.asarray(weights, dtype=np.float64)

    nc = _get_built()
    in_maps = []
    for c in range(NCORES):
        in_maps.append(
            {
                "y_pred": y_pred[c * S : (c + 1) * S],
                "y_true": y_true[c * S : (c + 1) * S],
            }
        )
    res = run_bass_kernel_spmd(nc, in_maps, list(range(NCORES)))
    sums = np.zeros(NTERMS, dtype=np.float64)
    for c in range(NCORES):
        sums += np.asarray(res.results[c]["out"], dtype=np.float64).sum(axis=0)

    A, Bs, C, D, E1, E2, SP0, SP1, SP0m, SP1m = sums
    F0 = E1 + 4.0 * SP0m
    F1 = E2 + 4.0 * SP1m
    Bn = float(B_TOTAL)
    evals_mse = (A + Bs) / (4.0 * Bn)
    mse_loss = (0.5 * A + 0.5 * C + D) / (3.0 * Bn)
    loss = (
        w[0] * evals_mse
        + w[1] * E1 / Bn
        + w[2] * E2 / Bn
        + w[3] * F1 / Bn
        + w[4] * F0 / Bn
        + w[5] * mse_loss
    )
    return np.float32(loss)



# revision 5
# speedup vs baseline: 1.4784x; 1.4784x over previous
"""Trainium2 Bass kernel for the EighMSE loss (data-parallel over 8 cores).

Math (replicates jax/LAPACK ssyevd eigenvector sign conventions for 2x2
symmetric matrices):
  row (a, b, c) encodes [[a, b], [b, c]]
  SM = a + c, DF = a - c, CD = c - a = -DF, RT = sqrt(DF^2 + 4 b^2)
  closed-form evals = (SM +- RT) / 2
  x = clamp(DF / RT, -1, 1); n0 = sqrt((1+x)/2), n1 = sqrt((1-x)/2)
  LAPACK larger-eval eigenvector g = (tau0*n0, tau1*n1) with
    tau0 = -1 if DF > 0 else sign(b)*sign(SM);  tau1 = tau0*sign(b)
  smaller-eval eigenvector = sign(SM) * (-g1, g0)

Implementation (choices driven by the TRN2 cost model: DVE bf16
tensor_tensor = 2x, tensor_scalar = 4x; gpsimd(Pool) supports only
tensor_copy; ACT `abs_reciprocal_sqrt_and_small` table provides 1/sqrt):
  - pred and true are processed as one packed [P, 2W] bf16 stream
    (halves [0:W] = pred, [W:2W] = true); pair terms use the halves.
  - a, b, c f32->bf16 conversions run on the otherwise-idle Pool engine.
  - Signs handled bitwise on uint16 views: signbit(t2) = signbit(CD) OR
    (signbit(b) XOR signbit(SM)) == signbit(tau0); signbit(tau1) adds
    signbit(b); pair products get their sign via XOR of sign masks.
  - xh = clamp(0.5*CD/RT, +-(0.5-2^-9)) so n0 = sqrt(0.5 - xh),
    n1 = sqrt(0.5 + xh); sum n0^2 = 0.5*N - sum xh (E-expansion):
      E1 = N - Sx - 2*sum(Q0),  E2 = N + Sx - 2*sum(Q1),
      Q0 = g0p*g0t = (n0p*n0t) ^ (parity of tau0 signs), etc.
    W0/W1 = sign(SMp)sign(SMt)*Q0/Q1; G = sum_{sx} Q = (SQ - SW)/2.
  - Bs = sum dRT^2 = (S4b + SCD2) - 2*sum sqrt(RTsq_p*RTsq_t).
  - ACT ops grouped per tile: [Square/ARS table] then [Sqrt table].

Per-core partial sums (NSTAT f32 per partition, summed on host):
  0: Sx   = sum xh (both sides)   1: S4b  = sum 4b^2 (both sides)
  2: SCD2 = sum CD^2 (both)       3: SQ0  4: SQ1  5: SW0  6: SW1
  7: A = sum dSM^2   8: C = sum dCD^2   9: Dm = sum db^2
  10: SRTx = sum RTp*RTt          11: spare
"""

import numpy as np
from contextlib import ExitStack

import concourse.bass as bass
import concourse.bacc as bacc
import concourse.tile as tile
from concourse import mybir
from concourse.bass_utils import run_bass_kernel_spmd

F32 = mybir.dt.float32
BF16 = mybir.dt.bfloat16
U16 = mybir.dt.uint16
OP = mybir.AluOpType
AF = mybir.ActivationFunctionType

B_TOTAL = 4_194_304
NCORES = 8
S = B_TOTAL // NCORES          # samples per core
P = 128                        # partitions
NPC = S // P                   # samples per partition (4096)
W = 1024                       # samples per tile per partition
NT = NPC // W                  # tiles per core
NSTAT = 12
XCLIP = 0.498046875            # 0.5 - 2^-9, bf16-exact; keeps sqrt args > 0

_BUILT = None
TRACE = False
LAST_RESULT = None


def _build_bass():
    nc = bacc.Bacc()
    yp = nc.declare_dram_parameter("y_pred", [S, 3], F32, isOutput=False)
    yt = nc.declare_dram_parameter("y_true", [S, 3], F32, isOutput=False)
    out = nc.declare_dram_parameter("out", [P, NSTAT], F32, isOutput=True)

    ypr = yp.rearrange("(p n) c -> p n c", p=P)
    ytr = yt.rearrange("(p n) c -> p n c", p=P)

    def xor_(o, a, b):
        nc.vector.tensor_tensor(
            o.bitcast(U16), a.bitcast(U16), b.bitcast(U16), op=OP.bitwise_xor
        )

    def or_(o, a, b):
        nc.vector.tensor_tensor(
            o.bitcast(U16), a.bitcast(U16), b.bitcast(U16), op=OP.bitwise_or
        )

    def andm(o, a):
        nc.vector.tensor_scalar(
            o.bitcast(U16), a.bitcast(U16), 0x8000, None, op0=OP.bitwise_and
        )

    with tile.TileContext(nc) as tc, ExitStack() as ctx:
        inp = ctx.enter_context(tc.tile_pool(name="inp", bufs=2))
        cvt = ctx.enter_context(tc.tile_pool(name="cvt", bufs=2))
        wk = ctx.enter_context(tc.tile_pool(name="wk", bufs=1))
        accp = ctx.enter_context(tc.tile_pool(name="accp", bufs=1))

        stats = accp.tile([P, NT * NSTAT], F32)
        nc.vector.memset(stats[:], 0.0)
        halfc = accp.tile([P, 1], F32)
        nc.vector.memset(halfc[:], 0.5)
        scrA = accp.tile([P, W], F32)   # ACT accumulate scratch
        scrV = accp.tile([P, W], BF16)  # DVE ts-accumulate scratch

        for it in range(NT):
            col = lambda k: stats[:, it * NSTAT + k : it * NSTAT + k + 1]

            xp = inp.tile([P, W, 3], F32, tag="xp")
            nc.sync.dma_start(xp[:], ypr[:, bass.ts(it, W), :])
            xt = inp.tile([P, W, 3], F32, tag="xt")
            nc.sync.dma_start(xt[:], ytr[:, bass.ts(it, W), :])

            # f32 -> packed bf16 [pred | true] on the Pool engine
            aB = cvt.tile([P, 2 * W], BF16, tag="aB")
            bB = cvt.tile([P, 2 * W], BF16, tag="bB")
            cB = cvt.tile([P, 2 * W], BF16, tag="cB")
            for h, x in ((0, xp), (1, xt)):
                sl = slice(h * W, (h + 1) * W)
                nc.gpsimd.tensor_copy(aB[:, sl], x[:, :, 0])
                nc.gpsimd.tensor_copy(bB[:, sl], x[:, :, 1])
                nc.gpsimd.tensor_copy(cB[:, sl], x[:, :, 2])

            # ---- packed chain over [P, 2W] ----
            SM = wk.tile([P, 2 * W], BF16, tag="SM")
            nc.vector.tensor_add(SM[:], aB[:], cB[:])
            CD = wk.tile([P, 2 * W], BF16, tag="CD")
            nc.vector.tensor_sub(CD[:], cB[:], aB[:])
            # ACT table B: Square + Abs_reciprocal_sqrt
            b4sq = wk.tile([P, 2 * W], BF16, tag="b4sq")
            nc.scalar.activation(b4sq[:], bB[:], AF.Square, scale=2.0,
                                 accum_out=col(1))
            CDsq = wk.tile([P, 2 * W], BF16, tag="CDsq")
            nc.scalar.activation(CDsq[:], CD[:], AF.Square, accum_out=col(2))
            RTsq = wk.tile([P, 2 * W], BF16, tag="RTsq")
            nc.vector.tensor_add(RTsq[:], b4sq[:], CDsq[:])
            rs = wk.tile([P, 2 * W], BF16, tag="rs")
            nc.scalar.activation(rs[:], RTsq[:], AF.Abs_reciprocal_sqrt)
            # xh = clamp(0.5 * CD / RT)
            xh = wk.tile([P, 2 * W], BF16, tag="xh")
            nc.vector.tensor_mul(xh[:], CD[:], rs[:])
            nc.vector.tensor_scalar(xh[:], xh[:], 0.5, XCLIP, op0=OP.mult, op1=OP.min)
            nc.vector.tensor_scalar(
                xh[:], xh[:], -XCLIP, None, op0=OP.max, op1=OP.add,
                accum_out=col(0),
            )
            # sign word: signbit = signbit(CD) | (signbit(b) ^ signbit(SM))
            t2 = wk.tile([P, 2 * W], BF16, tag="t2")
            xor_(t2[:], bB[:], SM[:])
            or_(t2[:], t2[:], CD[:])

            # ACT table A: Sqrt (n0/n1 for both sides at once)
            n0 = wk.tile([P, 2 * W], BF16, tag="n0")
            nc.scalar.activation(n0[:], xh[:], AF.Sqrt, scale=-1.0, bias=halfc[:])
            n1 = wk.tile([P, 2 * W], BF16, tag="n1")
            nc.scalar.activation(n1[:], xh[:], AF.Sqrt, scale=1.0, bias=halfc[:])

            # ---- pair terms on the halves ----
            hp = slice(0, W)
            ht = slice(W, 2 * W)

            def ts_sum(src, k):
                nc.vector.tensor_scalar(
                    scrV[:], src, 1.0, 0.0, op0=OP.mult, op1=OP.add,
                    accum_out=col(k),
                )

            Q0 = wk.tile([P, W], BF16, tag="Q0")
            nc.vector.tensor_mul(Q0[:], n0[:, hp], n0[:, ht])     # Pi0
            Q1 = wk.tile([P, W], BF16, tag="Q1")
            nc.vector.tensor_mul(Q1[:], n1[:, hp], n1[:, ht])     # Pi1
            e0 = wk.tile([P, W], BF16, tag="e0")
            xor_(e0[:], t2[:, hp], t2[:, ht])
            M0 = wk.tile([P, W], BF16, tag="M0")
            andm(M0[:], e0[:])
            xor_(Q0[:], Q0[:], M0[:])                             # Q0
            ts_sum(Q0[:], 3)
            v = wk.tile([P, W], BF16, tag="v")
            xor_(v[:], bB[:, hp], bB[:, ht])
            xor_(e0[:], e0[:], v[:])                              # e1
            andm(v[:], e0[:])                                     # M1 (reuse v)
            xor_(Q1[:], Q1[:], v[:])                              # Q1
            ts_sum(Q1[:], 4)
            w = wk.tile([P, W], BF16, tag="w")
            xor_(w[:], SM[:, hp], SM[:, ht])
            andm(w[:], w[:])                                      # mqm
            xor_(Q0[:], Q0[:], w[:])                              # W0
            ts_sum(Q0[:], 5)
            xor_(Q1[:], Q1[:], w[:])                              # W1
            ts_sum(Q1[:], 6)

            # squared diffs (ACT Square in table A) + RT cross term
            d0 = wk.tile([P, W], BF16, tag="d0")
            nc.vector.tensor_sub(d0[:], SM[:, hp], SM[:, ht])
            nc.scalar.activation(scrA[:], d0[:], AF.Square, accum_out=col(7))
            d1 = wk.tile([P, W], BF16, tag="d1")
            nc.vector.tensor_sub(d1[:], CD[:, hp], CD[:, ht])
            nc.scalar.activation(scrA[:], d1[:], AF.Square, accum_out=col(8))
            d2 = wk.tile([P, W], BF16, tag="d2")
            nc.vector.tensor_sub(d2[:], bB[:, hp], bB[:, ht])
            nc.scalar.activation(scrA[:], d2[:], AF.Square, accum_out=col(9))
            pi = wk.tile([P, W], BF16, tag="pi")
            nc.vector.tensor_mul(pi[:], RTsq[:, hp], RTsq[:, ht])
            nc.scalar.activation(scrA[:], pi[:], AF.Sqrt, accum_out=col(10))

        outsums = accp.tile([P, NSTAT], F32)
        stats3 = stats[:].rearrange("p (t k) -> p k t", t=NT)
        for k in range(NSTAT):
            nc.vector.tensor_reduce(
                outsums[:, k : k + 1], stats3[:, k, :],
                axis=mybir.AxisListType.X, op=OP.add,
            )
        nc.sync.dma_start(out[:, :], outsums[:])

    nc.compile()
    return nc


def _get_built():
    global _BUILT
    if _BUILT is None:
        _BUILT = _build_bass()
    return _BUILT


def kernel(y_pred: np.ndarray, y_true: np.ndarray, weights: np.ndarray) -> np.ndarray:
    global LAST_RESULT
    y_pred = np.ascontiguousarray(y_pred, dtype=np.float32)
    y_true = np.ascontiguousarray(y_true, dtype=np.float32)
    w = np.asarray(weights, dtype=np.float64)

    nc = _get_built()
    in_maps = []
    for c in range(NCORES):
        in_maps.append(
            {
                "y_pred": y_pred[c * S : (c + 1) * S],
                "y_true": y_true[c * S : (c + 1) * S],
            }
        )
    res = run_bass_kernel_spmd(nc, in_maps, list(range(NCORES)), trace=TRACE)
    LAST_RESULT = res
    sums = np.zeros(NSTAT, dtype=np.float64)
    for c in range(NCORES):
        sums += np.asarray(res.results[c]["out"], dtype=np.float64).sum(axis=0)

    Sx, S4b, SCD2, SQ0, SQ1, SW0, SW1, A, C, Dm, SRTx = sums[:11]
    N = float(B_TOTAL)
    E1 = N - Sx - 2.0 * SQ0
    E2 = N + Sx - 2.0 * SQ1
    G0 = 0.5 * (SQ0 - SW0)
    G1 = 0.5 * (SQ1 - SW1)
    F0 = E1 + 4.0 * G0
    F1 = E2 + 4.0 * G1
    Bs = (S4b + SCD2) - 2.0 * SRTx      # sum dRT^2
    evals_mse = (A + Bs) / (4.0 * N)
    mse_loss = (0.5 * A + 0.5 * C + Dm) / (3.0 * N)
    loss = (
        w[0] * evals_mse
        + w[1] * E1 / N
        + w[2] * E2 / N
        + w[3] * F1 / N
        + w[4] * F0 / N
        + w[5] * mse_loss
    )
    return np.float32(loss)
